# revision 19
# baseline (speedup 1.0000x reference)
"""Trainium2 Bass kernel for nn_CrossAttentionExpert.

Problem (hardcoded shapes): B=4, C=256, H=W=64 (N=4096), C8=32.
  cross_p2v = attn(q=wq_p@f_p, k=wk_v@f_v, v=wv_v@f_v)
  cross_v2p = attn(q=wq_v@f_v, k=wk_p@f_p, v=wv_p@f_p)
  out = BN(w_out @ concat([f_p, f_v, cross_p2v, cross_v2p]))  (training BN)

Sharding: 8 cores = (batch b, spatial half h).  Each core computes both
attention directions for its 2048 query positions (keys/values span all
4096 positions of its batch), the fused 1x1 output conv, and BN with a
[128,4] fp32 AllReduce of per-channel sum/sumsq across all 8 cores.

Layout: scores are computed transposed, S^T[n,m] (n=key on partitions,
m=query on free axis) so the exp'd probabilities feed the V^T matmul
moving operand directly -- no on-chip transposes.  Softmax skips the
max-subtraction (logits are O(25); exp fits fp32/bf16 range with huge
margin for this problem's 0.05-scaled weights); 1/rowsum is applied
after the V-matmul via a PE outer-product broadcast.

Perf structure (v7; 682us baseline -> ~370-390us measured, the spread
is cross-core throttle/collective jitter -- the compute phase is
~315us):
 - Everything on the PE is bf16 (inputs, weights, qr/kt/stg/vt/cross):
   1 cycle/row, 2x faster LDWEIGHTS, half the SBUF and half the input
   DMA.  End-to-end rel err ~8.9e-3 vs the 2e-2 gate (validated in
   numpy and on HW).
 - Software pipeline: AV matmuls of burst b-1 are emitted after the
   score matmuls of burst b, so the in-order PE queue never blocks on
   the ACT exp of the current burst.  Tail work (rowsum collapse,
   1/rowsum, cross muls), the fused output conv, and BN stat partials
   of tile t are emitted inside tile t+1's burst stream for the same
   reason.  av0 is double-buffered so the next (t,d)'s accumulation
   starts before the previous tail drains.
 - Rowsum: bf16 pairwise adds on DVE (t1/t2), fp32 burst accumulation
   on the Pool engine (NOT gpsimd tensor_scalar -- that lowers to DSP
   software and takes ~170us), final value rounded to bf16 so the
   collapse/broadcast matmuls run 1-cycle/row (f32r can't: codegen
   rejects degenerate-stationary f32r matmuls).  1/x via the custom-DVE
   reciprocal_approx_fast (5x faster than InstReciprocal).
 - PSUM budget (8 banks): pt 2x2 banks, av0 2, av1 1, misc 1.
 - All conv weights ship as ONE packed [128,3328] bf16 DMA on the
   scalar queue (kv1 follows it; kv0 on the sync queue), and convs are
   emitted in data-arrival order, so the PE starts ~0 and the whole
   6MB input load hides under the projection matmuls.  Key chunk j of
   burst b is 4*b+rg (burst <-> K-conv sub b, V-conv j 4b..4b+3), so
   the first attention bursts only need the first key quarter and can
   start while later quarters still stream in.
 - BN channel-sums ride the output-conv PSUM->SBUF copy via ACT
   accum_out; sum-of-squares is a DVE scalar_tensor_tensor per m-tile.
   A dummy warmup AllReduce absorbs the collective's first-use cost;
   the real [128,4] AllReduce (~25us, latency-bound) is the only
   exposed tail besides normalize+writeback.
 - Known limits: the PE duty-cycle throttles to ~1.2GHz under
   sustained load (util_limit 50%), which caps the AV+scores stream at
   ~290us busy; fp8 DoubleRow would halve AV but needs a per-query max
   subtraction whose cheap bounds are too loose (validated: NaN/6%+
   error in numpy), so it is not used.
"""

import numpy as np

import concourse.bass as bass
import concourse.mybir as mybir
import concourse.tile as tile
from concourse import bacc, bass_utils

FP = mybir.dt.float32
FR = mybir.dt.float32r
BF = mybir.dt.bfloat16
P = 128
C = 256
C8 = 32
N = 4096          # full spatial positions per batch
M = 2048          # local query positions per core
NMT = 4           # m-tiles of 512
MT = 512
NCORES = 8
BN_EPS = 1e-5
BN_COUNT = 4 * 4096  # B * H * W

_ALU = mybir.AluOpType
_ACT = mybir.ActivationFunctionType

_PROGRAM = None


def _build_program():
    nc = bacc.Bacc("TRN2", target_bir_lowering=False, debug=False,
                   num_devices=NCORES)

    # ---- DRAM I/O ----
    kv = [nc.dram_tensor(f"kv{d}", [C, N], BF, kind="ExternalInput").ap()
          for d in range(2)]
    # all conv weights packed into one [P, 3328] bf16 tensor (one DMA):
    # cols: wq0(64) wq1(64) wk0(64) wk1(64) wv0(512) wv1(512)
    #       wout(1024) woutc(1024)
    wpack = nc.dram_tensor("wpack", [P, 3328], BF,
                           kind="ExternalInput").ap()
    bpack = nc.dram_tensor("bpack", [P, 10], FP, kind="ExternalInput").ap()
    yout = nc.dram_tensor("y", [C, M], FP, kind="ExternalOutput").ap()

    with tile.TileContext(nc) as tc:
        with (
            nc.allow_low_precision(
                reason="bf16 attention intermediates; "
                       "end-to-end rel err ~4.5e-3 vs 2e-2 gate"),
            tc.tile_pool(name="consts", bufs=1) as consts,
            tc.tile_pool(name="big", bufs=1) as big,
            tc.tile_pool(name="vt", bufs=64) as vtp,
            tc.tile_pool(name="st", bufs=3) as stp,
            tc.tile_pool(name="rs", bufs=3) as rsp,
            tc.tile_pool(name="cross", bufs=4) as p_cross,
            tc.tile_pool(name="small", bufs=4) as p_small,
            tc.tile_pool(name="psA", bufs=2, space="PSUM") as psA,
            tc.tile_pool(name="psB", bufs=1, space="PSUM") as psB,
            tc.tile_pool(name="psC", bufs=1, space="PSUM") as psC,
            tc.tile_pool(name="dram", bufs=1, space="DRAM") as dram,
        ):
            # ---- all weights in one DMA, biases in another ----
            wpack_sb = consts.tile([P, 3328], BF, name="wpacksb")
            nc.scalar.dma_start(wpack_sb[:], wpack[:])
            bpack_sb = consts.tile([P, 10], FP, name="bpacksb")
            nc.scalar.dma_start(bpack_sb[:], bpack[:])

            def wview(off, o, m):
                return wpack_sb[:, off:off + o * m].rearrange(
                    "p (o m) -> p o m", o=o)

            wq_sb = [wview(64 * d, 2, C8) for d in range(2)]
            wk_sb = [wview(128 + 64 * d, 2, C8) for d in range(2)]
            wv_sb = [wview(256 + 512 * d, 2, C) for d in range(2)]
            wout_sb = wview(1280, 4, C)
            woutc_sb = wview(2304, 4, C)
            biasq_sb = bpack_sb[:, 0:4]
            ybias_sb = bpack_sb[:, 4:6]
            gb_sb = bpack_sb[:, 6:10]

            ones_col = consts.tile([P, 1], BF, name="ones_col")
            nc.vector.memset(ones_col[:], 1.0)
            ones_row = consts.tile([1, P], BF, name="ones_row")
            nc.vector.memset(ones_row[:], 1.0)

            # ---- kv loads, position-major so convs can start early ----
            kv_sb = [big.tile([P, 2, N], BF, name=f"kvsb{d}")
                     for d in range(2)]
            for d in range(2):
                src = kv[d].rearrange("(o p) n -> p o n", p=P)
                eng = nc.sync if d == 0 else nc.scalar
                for h in range(2):
                    sl = slice(h * M, (h + 1) * M)
                    for o in range(2):
                        eng.dma_start(kv_sb[d][:, o, sl], src[:, o, sl])

            # ---- persistent activations ----
            qr = [big.tile([32, M], BF, name=f"qr{d}") for d in range(2)]
            kt = [big.tile([32, N], BF, name=f"kt{d}") for d in range(2)]
            y_acc = [big.tile([P, M], FP, name=f"yacc{cc}") for cc in range(2)]
            vt = [[], []]

            # ---- projections, in DMA-arrival order ----
            # dir0: q from kv1 (f_p), k/v from kv0 (f_v); dir1 swapped.
            def k_conv(d, subs):
                kkv = kv_sb[d]
                for sub in subs:
                    nsl = slice(sub * MT, (sub + 1) * MT)
                    ps = psA.tile([32, MT], FP, tag="pt", name="kps")
                    for kc in range(2):
                        nc.tensor.matmul(
                            ps, wk_sb[d][:, kc, :], kkv[:, kc, nsl],
                            start=(kc == 0), stop=(kc == 1))
                    nc.vector.tensor_scalar_add(
                        kt[d][:, nsl], ps,
                        biasq_sb[0:32, 2 * d + 1:2 * d + 2])

            def v_conv(d, js):
                kkv = kv_sb[d]
                for j in js:
                    ps = psA.tile([P, C], FP, tag="pt", name="vps")
                    for kc in range(2):
                        nc.tensor.matmul(
                            ps, kkv[:, kc, j * P:(j + 1) * P],
                            wv_sb[d][:, kc, :],
                            start=(kc == 0), stop=(kc == 1))
                    v = vtp.tile([P, C], BF, tag="vt", name="vtt")
                    nc.vector.tensor_copy(v[:], ps)
                    vt[d].append(v)

            def q_conv(d):
                qkv = kv_sb[1 - d]
                for t in range(NMT):
                    msl = slice(t * MT, (t + 1) * MT)
                    ps = psA.tile([32, MT], FP, tag="pt", name="qps")
                    for kc in range(2):
                        nc.tensor.matmul(
                            ps, wq_sb[d][:, kc, :], qkv[:, kc, msl],
                            start=(kc == 0), stop=(kc == 1))
                    nc.vector.tensor_scalar_add(
                        qr[d][:, msl], ps,
                        biasq_sb[0:32, 2 * d:2 * d + 1])

            k_conv(0, [0, 1]); v_conv(0, range(0, 8))
            k_conv(0, [2, 3]); v_conv(0, range(8, 16))
            q_conv(1)          # reads kv0 first half
            q_conv(0)          # reads kv1 first half
            k_conv(0, [4, 5]); v_conv(0, range(16, 24))
            k_conv(0, [6, 7]); v_conv(0, range(24, 32))
            for h in range(2):
                k_conv(1, range(4 * h, 4 * h + 4))
                v_conv(1, range(16 * h, 16 * h + 16))

            # ---- BN stat partials, accumulated per m-tile ----
            ssum = p_small.tile([P, 2, NMT], FP, tag="ssum", name="ssum")
            ssq = p_small.tile([P, 2, NMT], FP, tag="ssq", name="ssq")

            # ---- attention + fused output conv ----
            # Tail/yc/stats of tile t are emitted inside tile t+1's burst
            # stream so the in-order PE queue never blocks on them.
            crs_all = {}

            def make_tail(t, d, av, racc_bf):
                def emit():
                    rsum_ps = psC.tile([1, MT], FP, tag="misc", name="rsum")
                    nc.tensor.matmul(rsum_ps, ones_col[:], racc_bf[:],
                                     start=True, stop=True)
                    rs_bf = p_small.tile([1, MT], BF, tag="rsbf",
                                         name="rsbf")
                    nc.vector.tensor_copy(rs_bf[:], rsum_ps)
                    rbc_ps = psC.tile([P, MT], FP, tag="misc", name="rbc")
                    nc.tensor.matmul(rbc_ps, ones_row[:], rs_bf[:],
                                     start=True, stop=True)
                    rbc = p_cross.tile([P, MT], FP, tag="rbc", name="rbc_sb")
                    nc.vector.reciprocal_approx_fast(out=rbc[:], in_=rbc_ps)
                    for cc in range(2):
                        cross = p_cross.tile([P, MT], BF, tag="cross",
                                             name="cross")
                        nc.vector.tensor_mul(cross[:], av[cc], rbc[:])
                        crs_all[(t, d, cc)] = cross
                return emit

            def make_yc(t):
                msl = slice(t * MT, (t + 1) * MT)

                def emit():
                    for oc in range(2):
                        ocs = slice(oc * P, (oc + 1) * P)
                        yc = psC.tile([P, MT], FP, tag="misc", name="yc")
                        nc.tensor.matmul(yc, wout_sb[:, 0, ocs],
                                         kv_sb[1][:, 0, msl],
                                         start=True, stop=False)
                        nc.tensor.matmul(yc, wout_sb[:, 1, ocs],
                                         kv_sb[1][:, 1, msl],
                                         start=False, stop=False)
                        nc.tensor.matmul(yc, wout_sb[:, 2, ocs],
                                         kv_sb[0][:, 0, msl],
                                         start=False, stop=False)
                        nc.tensor.matmul(yc, wout_sb[:, 3, ocs],
                                         kv_sb[0][:, 1, msl],
                                         start=False, stop=False)
                        for d in range(2):
                            for cc in range(2):
                                nc.tensor.matmul(
                                    yc, woutc_sb[:, 2 * d + cc, ocs],
                                    crs_all[(t, d, cc)][:],
                                    start=False,
                                    stop=(d == 1 and cc == 1))
                        nc.scalar.activation(y_acc[oc][:, msl], yc,
                                             _ACT.Identity,
                                             bias=ybias_sb[:, oc:oc + 1],
                                             accum_out=ssum[:, oc, t:t + 1])
                return emit

            def make_stats(t):
                msl = slice(t * MT, (t + 1) * MT)

                def emit():
                    for cc in range(2):
                        sq = p_small.tile([P, MT], BF, tag="sq", name="sq",
                                          bufs=2)
                        nc.vector.scalar_tensor_tensor(
                            out=sq[:], in0=y_acc[cc][:, msl], scalar=1.0,
                            in1=y_acc[cc][:, msl],
                            op0=_ALU.mult, op1=_ALU.mult,
                            accum_out=ssq[:, cc, t:t + 1])
                return emit

            pend_tail = pend_yc = pend_stats = None
            for t in range(NMT):
                msl = slice(t * MT, (t + 1) * MT)
                for d in range(2):
                    av = [psB.tile([P, MT], FP, tag=f"av{i}", name=f"av{i}",
                                   bufs=2 - i) for i in range(2)]
                    racc = rsp.tile([P, MT], FP, tag="racc", name="racc")
                    racc_bf = rsp.tile([P, MT], BF, tag="raccbf",
                                       name="racc_bf")
                    stg_q = [None] * 8
                    for bb in range(9):
                        if bb < 8:
                            # scores + exp for burst bb
                            stg = stp.tile([P, 4 * MT], BF, tag="st",
                                           name="stg")
                            stg_q[bb] = stg
                            for half in range(2):
                                pt = psA.tile([P, 2, MT], FP, tag="pt",
                                              name="pt")
                                for rr in range(2):
                                    rg = 2 * half + rr
                                    kj = 4 * bb + rg
                                    ksl = slice(kj * P, (kj + 1) * P)
                                    nc.tensor.matmul(
                                        pt[:, rr, :], kt[d][:, ksl],
                                        qr[d][:, msl],
                                        start=True, stop=True)
                                nc.scalar.activation(
                                    stg[:, half * 1024:(half + 1) * 1024],
                                    pt[:, :, :], _ACT.Exp)
                        if bb == 0 and pend_tail is not None:
                            pend_tail()
                            pend_tail = None
                        if bb == 1 and pend_yc is not None:
                            pend_yc()
                            pend_yc = None
                        if bb == 2 and pend_stats is not None:
                            pend_stats()
                            pend_stats = None
                        if bb >= 1:
                            # AV + rowsum for burst bb-1
                            b = bb - 1
                            stg = stg_q[b]
                            for rg in range(4):
                                j = 4 * b + rg
                                ssl = slice(rg * MT, (rg + 1) * MT)
                                for cc in range(2):
                                    nc.tensor.matmul(
                                        av[cc],
                                        vt[d][j][:, cc * P:(cc + 1) * P],
                                        stg[:, ssl],
                                        start=(b == 0 and rg == 0),
                                        stop=(b == 7 and rg == 3))
                            t1 = rsp.tile([P, 2 * MT], BF, tag="t1",
                                          name="t1")
                            nc.vector.tensor_add(t1[:], stg[:, 0:1024],
                                                 stg[:, 1024:2048])
                            t2 = rsp.tile([P, MT], BF, tag="t2", name="t2")
                            nc.vector.tensor_add(t2[:], t1[:, 0:MT],
                                                 t1[:, MT:2 * MT])
                            if b == 0:
                                nc.gpsimd.tensor_copy(racc[:], t2[:])
                            else:
                                nc.gpsimd.tensor_add(
                                    racc_bf[:] if b == 7 else racc[:],
                                    racc[:], t2[:])
                    pend_tail = make_tail(t, d, av, racc_bf)
                pend_yc = make_yc(t)
                pend_stats = make_stats(t)

            pend_tail()
            pend_yc()
            pend_stats()

            # ---- BN: collapse partials, AllReduce, normalize ----
            stats = p_small.tile([P, 4], FP, tag="stats", name="stats")
            nc.vector.reduce_sum(stats[:, 0:2], ssum[:],
                                 axis=mybir.AxisListType.X)
            nc.vector.reduce_sum(stats[:, 2:4], ssq[:],
                                 axis=mybir.AxisListType.X)
            cc_in = dram.tile([P, 4], FP, name="cc_in")
            cc_out = dram.tile([P, 4], FP, name="cc_out")
            nc.sync.dma_start(cc_in[:], stats[:])
            nc.gpsimd.collective_compute(
                "AllReduce", _ALU.add,
                replica_groups=[list(range(NCORES))],
                ins=[cc_in.opt()], outs=[cc_out.opt()])
            ar = p_small.tile([P, 4], FP, tag="ar", name="ar")
            nc.sync.dma_start(ar[:], cc_out[:])

            inv_n = 1.0 / BN_COUNT
            yo = yout.rearrange("(o p) m -> p o m", p=P)
            scales, shifts = [], []
            for cc in range(2):
                mean = p_small.tile([P, 1], FP, tag="bn", name="mean")
                ex2 = p_small.tile([P, 1], FP, tag="bn", name="ex2")
                var = p_small.tile([P, 1], FP, tag="bn", name="var")
                nc.vector.tensor_scalar_mul(mean[:], ar[:, cc:cc + 1], inv_n)
                nc.vector.tensor_scalar_mul(ex2[:], ar[:, 2 + cc:3 + cc],
                                            inv_n)
                nc.vector.tensor_tensor(var[:], mean[:], mean[:], _ALU.mult)
                nc.vector.tensor_sub(var[:], ex2[:], var[:])
                sd = p_small.tile([P, 1], FP, tag="bn", name="sd")
                nc.vector.tensor_scalar_add(var[:], var[:], BN_EPS)
                nc.scalar.activation(sd[:], var[:], _ACT.Sqrt)
                rstd = p_small.tile([P, 1], FP, tag="bn", name="rstd")
                nc.vector.reciprocal(rstd[:], sd[:])
                scale = p_small.tile([P, 1], FP, tag="bnp", name="scale")
                nc.vector.tensor_tensor(scale[:], gb_sb[:, cc:cc + 1],
                                        rstd[:], _ALU.mult)
                shift = p_small.tile([P, 1], FP, tag="bnp", name="shift")
                nc.vector.tensor_tensor(shift[:], mean[:], scale[:],
                                        _ALU.mult)
                nc.vector.tensor_sub(shift[:], gb_sb[:, 2 + cc:3 + cc],
                                     shift[:])
                scales.append(scale)
                shifts.append(shift)
            for q in range(2):
                qsl = slice(q * 1024, (q + 1) * 1024)
                for cc in range(2):
                    nc.vector.tensor_scalar(
                        out=y_acc[cc][:, qsl], in0=y_acc[cc][:, qsl],
                        scalar1=scales[cc][:], scalar2=shifts[cc][:],
                        op0=_ALU.mult, op1=_ALU.add)
                    eng = nc.sync if cc == 0 else nc.scalar
                    eng.dma_start(yo[:, cc, qsl], y_acc[cc][:, qsl])

    nc.compile()
    return nc


def _get_program():
    global _PROGRAM
    if _PROGRAM is None:
        _PROGRAM = _build_program()
    return _PROGRAM


def _make_in_maps(inputs):
    BF_NP = mybir.dt.np(mybir.dt.bfloat16)
    f_p = np.ascontiguousarray(
        np.asarray(inputs["f_p"], np.float32).reshape(4, C, N))
    f_v = np.ascontiguousarray(
        np.asarray(inputs["f_v"], np.float32).reshape(4, C, N))

    def T(x):
        return np.ascontiguousarray(np.asarray(x, np.float32).T)

    w_out = np.asarray(inputs["w_out"], np.float32)
    bv_v = np.asarray(inputs["bv_v"], np.float32)
    bv_p = np.asarray(inputs["bv_p"], np.float32)
    # wv-bias terms of the cross contributions, folded into one vector.
    yb = w_out[:, 2 * C:3 * C] @ bv_v + w_out[:, 3 * C:] @ bv_p
    def pack_w(x, o):
        # [C_in, m] transposed weight -> per-partition [P, o*m] block
        t = T(x).astype(np.float32).reshape(o, P, -1).transpose(1, 0, 2)
        return t.reshape(P, -1)

    wcols = [
        pack_w(inputs["wq_p"], 2), pack_w(inputs["wq_v"], 2),
        pack_w(inputs["wk_v"], 2), pack_w(inputs["wk_p"], 2),
        pack_w(inputs["wv_v"], 2), pack_w(inputs["wv_p"], 2),
        pack_w(w_out[:, :2 * C], 4), pack_w(w_out[:, 2 * C:], 4),
    ]
    wpack = np.concatenate(wcols, axis=1).astype(BF_NP)
    biasq = np.stack(
        [np.tile(np.asarray(inputs[k], np.float32), 4)
         for k in ("bq_p", "bk_v", "bq_v", "bk_p")], axis=1)
    gamma = np.asarray(inputs["gamma"], np.float32)
    beta = np.asarray(inputs["beta"], np.float32)
    bpack = np.concatenate(
        [biasq, np.stack([yb[:P], yb[P:]], axis=1),
         np.stack([gamma[:P], gamma[P:], beta[:P], beta[P:]], axis=1)],
        axis=1).astype(np.float32)
    shared = {
        "wpack": np.ascontiguousarray(wpack),
        "bpack": np.ascontiguousarray(bpack),
    }
    in_maps = []
    for core in range(NCORES):
        b, h = divmod(core, 2)
        # roll so this core's query half sits at columns [0, 2048); K/V use
        # the full (permuted) range -- softmax/AV are key-order-invariant.
        kv1 = np.ascontiguousarray(
            np.roll(f_p[b], -h * M, axis=1).astype(BF_NP))
        kv0 = np.ascontiguousarray(
            np.roll(f_v[b], -h * M, axis=1).astype(BF_NP))
        in_maps.append({"kv0": kv0, "kv1": kv1, **shared})
    return in_maps


def _assemble(results):
    out = np.empty((4, C, N), np.float32)
    for core in range(NCORES):
        b, h = divmod(core, 2)
        out[b][:, h * M:(h + 1) * M] = results[core]["y"]
    return out.reshape(4, C, 64, 64)


def _run(inputs, **kwargs):
    nc = _get_program()
    in_maps = _make_in_maps(inputs)
    res = bass_utils.run_bass_kernel_spmd(
        nc, in_maps, core_ids=list(range(NCORES)), **kwargs)
    return _assemble(res.results), res


def kernel(**inputs):
    out, _ = _run(inputs)
    return out


# revision 20
# speedup vs baseline: 1.0956x; 1.0956x over previous
"""Trainium2 Bass kernel for nn_CrossAttentionExpert.

Problem (hardcoded shapes): B=4, C=256, H=W=64 (N=4096), C8=32.
  cross_p2v = attn(q=wq_p@f_p, k=wk_v@f_v, v=wv_v@f_v)
  cross_v2p = attn(q=wq_v@f_v, k=wk_p@f_p, v=wv_p@f_p)
  out = BN(w_out @ concat([f_p, f_v, cross_p2v, cross_v2p]))  (training BN)

Sharding: 8 cores = (batch b, spatial half h).  Each core computes both
attention directions for its 2048 query positions (keys/values span all
4096 positions of its batch), the fused 1x1 output conv, and BN with a
[128,4] fp32 AllReduce of per-channel sum/sumsq across all 8 cores.

Layout: scores are computed transposed, S^T[n,m] (n=key on partitions,
m=query on free axis) so the exp'd probabilities feed the V^T matmul
moving operand directly -- no on-chip transposes.  Softmax skips the
max-subtraction (logits are O(25); exp fits fp32/bf16 range with huge
margin for this problem's 0.05-scaled weights); 1/rowsum is applied
after the V-matmul via a PE outer-product broadcast.

Perf structure (v7; 682us baseline -> ~370-390us measured, the spread
is cross-core throttle/collective jitter -- the compute phase is
~315us):
 - Everything on the PE is bf16 (inputs, weights, qr/kt/stg/vt/cross):
   1 cycle/row, 2x faster LDWEIGHTS, half the SBUF and half the input
   DMA.  End-to-end rel err ~8.9e-3 vs the 2e-2 gate (validated in
   numpy and on HW).
 - Software pipeline: AV matmuls of burst b-1 are emitted after the
   score matmuls of burst b, so the in-order PE queue never blocks on
   the ACT exp of the current burst.  Tail work (rowsum collapse,
   1/rowsum, cross muls), the fused output conv, and BN stat partials
   of tile t are emitted inside tile t+1's burst stream for the same
   reason.  av0 is double-buffered so the next (t,d)'s accumulation
   starts before the previous tail drains.
 - Rowsum: bf16 pairwise adds on DVE (t1/t2), fp32 burst accumulation
   on the Pool engine (NOT gpsimd tensor_scalar -- that lowers to DSP
   software and takes ~170us), final value rounded to bf16 so the
   collapse/broadcast matmuls run 1-cycle/row (f32r can't: codegen
   rejects degenerate-stationary f32r matmuls).  1/x via the custom-DVE
   reciprocal_approx_fast (5x faster than InstReciprocal).
 - PSUM budget (8 banks): pt 2x2 banks, av0 2, av1 1, misc 1.
 - All conv weights ship as ONE packed [128,3328] bf16 DMA on the
   scalar queue (kv1 follows it; kv0 on the sync queue), and convs are
   emitted in data-arrival order, so the PE starts ~0 and the whole
   6MB input load hides under the projection matmuls.  Key chunk j of
   burst b is 4*b+rg (burst <-> K-conv sub b, V-conv j 4b..4b+3), so
   the first attention bursts only need the first key quarter and can
   start while later quarters still stream in.
 - BN channel-sums ride the output-conv PSUM->SBUF copy via ACT
   accum_out; sum-of-squares is a DVE scalar_tensor_tensor per m-tile.
   A dummy warmup AllReduce absorbs the collective's first-use cost;
   the real [128,4] AllReduce (~25us, latency-bound) is the only
   exposed tail besides normalize+writeback.
 - Known limits: the PE duty-cycle throttles to ~1.2GHz under
   sustained load (util_limit 50%), which caps the AV+scores stream at
   ~290us busy; fp8 DoubleRow would halve AV but needs a per-query max
   subtraction whose cheap bounds are too loose (validated: NaN/6%+
   error in numpy), so it is not used.
"""

import numpy as np

import concourse.bass as bass
import concourse.mybir as mybir
import concourse.tile as tile
from concourse import bacc, bass_utils

FP = mybir.dt.float32
FR = mybir.dt.float32r
BF = mybir.dt.bfloat16
P = 128
C = 256
C8 = 32
N = 4096          # full spatial positions per batch
M = 2048          # local query positions per core
NMT = 4           # m-tiles of 512
MT = 512
NCORES = 8
BN_EPS = 1e-5
BN_COUNT = 4 * 4096  # B * H * W

_ALU = mybir.AluOpType
_ACT = mybir.ActivationFunctionType

_PROGRAM = None


def _build_program():
    nc = bacc.Bacc("TRN2", target_bir_lowering=False, debug=False,
                   num_devices=NCORES)

    # ---- DRAM I/O ----
    kv = [nc.dram_tensor(f"kv{d}", [C, N], BF, kind="ExternalInput").ap()
          for d in range(2)]
    # all conv weights packed into one [P, 3328] bf16 tensor (one DMA):
    # cols: wq0(64) wq1(64) wk0(64) wk1(64) wv0(512) wv1(512)
    #       wout(1024) woutc(1024)
    wpack = nc.dram_tensor("wpack", [P, 3328], BF,
                           kind="ExternalInput").ap()
    bpack = nc.dram_tensor("bpack", [P, 10], FP, kind="ExternalInput").ap()
    yout = nc.dram_tensor("y", [C, M], FP, kind="ExternalOutput").ap()

    with tile.TileContext(nc) as tc:
        with (
            nc.allow_low_precision(
                reason="bf16 attention intermediates; "
                       "end-to-end rel err ~4.5e-3 vs 2e-2 gate"),
            tc.tile_pool(name="consts", bufs=1) as consts,
            tc.tile_pool(name="big", bufs=1) as big,
            tc.tile_pool(name="vt", bufs=64) as vtp,
            tc.tile_pool(name="st", bufs=3) as stp,
            tc.tile_pool(name="rs", bufs=3) as rsp,
            tc.tile_pool(name="cross", bufs=4) as p_cross,
            tc.tile_pool(name="small", bufs=4) as p_small,
            tc.tile_pool(name="psA", bufs=2, space="PSUM") as psA,
            tc.tile_pool(name="psB", bufs=1, space="PSUM") as psB,
            tc.tile_pool(name="psC", bufs=1, space="PSUM") as psC,
            tc.tile_pool(name="dram", bufs=1, space="DRAM") as dram,
        ):
            # ---- all weights in one DMA, biases in another ----
            wpack_sb = consts.tile([P, 3328], BF, name="wpacksb")
            nc.scalar.dma_start(wpack_sb[:], wpack[:])
            bpack_sb = consts.tile([P, 10], FP, name="bpacksb")
            nc.scalar.dma_start(bpack_sb[:], bpack[:])

            def wview(off, o, m):
                return wpack_sb[:, off:off + o * m].rearrange(
                    "p (o m) -> p o m", o=o)

            wq_sb = [wview(64 * d, 2, C8) for d in range(2)]
            wk_sb = [wview(128 + 64 * d, 2, C8) for d in range(2)]
            wv_sb = [wview(256 + 512 * d, 2, C) for d in range(2)]
            wout_sb = wview(1280, 4, C)
            woutc_sb = wview(2304, 4, C)
            biasq_sb = bpack_sb[:, 0:4]
            ybias_sb = bpack_sb[:, 4:6]
            gb_sb = bpack_sb[:, 6:10]

            ones_col = consts.tile([P, 1], BF, name="ones_col")
            nc.vector.memset(ones_col[:], 1.0)
            ones_row = consts.tile([1, P], BF, name="ones_row")
            nc.vector.memset(ones_row[:], 1.0)

            # ---- kv loads, position-major so convs can start early ----
            kv_sb = [big.tile([P, 2, N], BF, name=f"kvsb{d}")
                     for d in range(2)]
            for d in range(2):
                src = kv[d].rearrange("(o p) n -> p o n", p=P)
                eng = nc.sync if d == 0 else nc.scalar
                for h in range(2):
                    sl = slice(h * M, (h + 1) * M)
                    for o in range(2):
                        eng.dma_start(kv_sb[d][:, o, sl], src[:, o, sl])

            # warm up the collective path while convs run; result unused.
            warm_in = dram.tile([P, 4], FP, name="warm_in")
            warm_out = dram.tile([P, 4], FP, name="warm_out")
            nc.gpsimd.collective_compute(
                "AllReduce", _ALU.add,
                replica_groups=[list(range(NCORES))],
                ins=[warm_in.opt()], outs=[warm_out.opt()])

            # ---- persistent activations ----
            qr = [big.tile([32, M], BF, name=f"qr{d}") for d in range(2)]
            kt = [big.tile([32, N], BF, name=f"kt{d}") for d in range(2)]
            y_acc = [big.tile([P, M], FP, name=f"yacc{cc}") for cc in range(2)]
            vt = [[], []]

            # ---- projections, in DMA-arrival order ----
            # dir0: q from kv1 (f_p), k/v from kv0 (f_v); dir1 swapped.
            def k_conv(d, subs):
                kkv = kv_sb[d]
                for sub in subs:
                    nsl = slice(sub * MT, (sub + 1) * MT)
                    ps = psA.tile([32, MT], FP, tag="pt", name="kps")
                    for kc in range(2):
                        nc.tensor.matmul(
                            ps, wk_sb[d][:, kc, :], kkv[:, kc, nsl],
                            start=(kc == 0), stop=(kc == 1))
                    nc.scalar.activation(
                        kt[d][:, nsl], ps, _ACT.Identity,
                        bias=biasq_sb[0:32, 2 * d + 1:2 * d + 2])

            def v_conv(d, js):
                kkv = kv_sb[d]
                for j in js:
                    ps = psA.tile([P, C], FP, tag="pt", name="vps")
                    for kc in range(2):
                        nc.tensor.matmul(
                            ps, kkv[:, kc, j * P:(j + 1) * P],
                            wv_sb[d][:, kc, :],
                            start=(kc == 0), stop=(kc == 1))
                    v = vtp.tile([P, C], BF, tag="vt", name="vtt")
                    nc.vector.tensor_copy(v[:], ps)
                    vt[d].append(v)

            def q_conv(d):
                qkv = kv_sb[1 - d]
                for t in range(NMT):
                    msl = slice(t * MT, (t + 1) * MT)
                    ps = psA.tile([32, MT], FP, tag="pt", name="qps")
                    for kc in range(2):
                        nc.tensor.matmul(
                            ps, wq_sb[d][:, kc, :], qkv[:, kc, msl],
                            start=(kc == 0), stop=(kc == 1))
                    nc.scalar.activation(qr[d][:, msl], ps, _ACT.Identity,
                                         bias=biasq_sb[0:32, 2 * d:2 * d + 1])

            k_conv(0, [0, 1]); v_conv(0, range(0, 8))
            k_conv(0, [2, 3]); v_conv(0, range(8, 16))
            q_conv(1)          # reads kv0 first half
            q_conv(0)          # reads kv1 first half
            k_conv(0, [4, 5]); v_conv(0, range(16, 24))
            k_conv(0, [6, 7]); v_conv(0, range(24, 32))
            for h in range(2):
                k_conv(1, range(4 * h, 4 * h + 4))
                v_conv(1, range(16 * h, 16 * h + 16))

            # ---- BN stat partials, accumulated per m-tile ----
            ssum = p_small.tile([P, 2, NMT], FP, tag="ssum", name="ssum")
            ssq = p_small.tile([P, 2, NMT], FP, tag="ssq", name="ssq")

            # ---- attention + fused output conv ----
            # Tail/yc/stats of tile t are emitted inside tile t+1's burst
            # stream so the in-order PE queue never blocks on them.
            crs_all = {}

            def make_tail(t, d, av, racc_bf):
                def emit():
                    rsum_ps = psC.tile([1, MT], FP, tag="misc", name="rsum")
                    nc.tensor.matmul(rsum_ps, ones_col[:], racc_bf[:],
                                     start=True, stop=True)
                    rs_bf = p_small.tile([1, MT], BF, tag="rsbf",
                                         name="rsbf")
                    nc.vector.tensor_copy(rs_bf[:], rsum_ps)
                    rbc_ps = psC.tile([P, MT], FP, tag="misc", name="rbc")
                    nc.tensor.matmul(rbc_ps, ones_row[:], rs_bf[:],
                                     start=True, stop=True)
                    rbc = p_cross.tile([P, MT], FP, tag="rbc", name="rbc_sb")
                    nc.vector.reciprocal_approx_fast(out=rbc[:], in_=rbc_ps)
                    for cc in range(2):
                        cross = p_cross.tile([P, MT], BF, tag="cross",
                                             name="cross")
                        nc.vector.tensor_mul(cross[:], av[cc], rbc[:])
                        crs_all[(t, d, cc)] = cross
                return emit

            def make_yc(t):
                msl = slice(t * MT, (t + 1) * MT)

                def emit():
                    for oc in range(2):
                        ocs = slice(oc * P, (oc + 1) * P)
                        yc = psC.tile([P, MT], FP, tag="misc", name="yc")
                        nc.tensor.matmul(yc, wout_sb[:, 0, ocs],
                                         kv_sb[1][:, 0, msl],
                                         start=True, stop=False)
                        nc.tensor.matmul(yc, wout_sb[:, 1, ocs],
                                         kv_sb[1][:, 1, msl],
                                         start=False, stop=False)
                        nc.tensor.matmul(yc, wout_sb[:, 2, ocs],
                                         kv_sb[0][:, 0, msl],
                                         start=False, stop=False)
                        nc.tensor.matmul(yc, wout_sb[:, 3, ocs],
                                         kv_sb[0][:, 1, msl],
                                         start=False, stop=False)
                        for d in range(2):
                            for cc in range(2):
                                nc.tensor.matmul(
                                    yc, woutc_sb[:, 2 * d + cc, ocs],
                                    crs_all[(t, d, cc)][:],
                                    start=False,
                                    stop=(d == 1 and cc == 1))
                        nc.scalar.activation(y_acc[oc][:, msl], yc,
                                             _ACT.Identity,
                                             bias=ybias_sb[:, oc:oc + 1],
                                             accum_out=ssum[:, oc, t:t + 1])
                return emit

            def make_stats(t):
                msl = slice(t * MT, (t + 1) * MT)

                def emit():
                    for cc in range(2):
                        sq = p_small.tile([P, MT], BF, tag="sq", name="sq",
                                          bufs=2)
                        nc.vector.scalar_tensor_tensor(
                            out=sq[:], in0=y_acc[cc][:, msl], scalar=1.0,
                            in1=y_acc[cc][:, msl],
                            op0=_ALU.mult, op1=_ALU.mult,
                            accum_out=ssq[:, cc, t:t + 1])
                return emit

            pend_tail = pend_yc = pend_stats = None
            for t in range(NMT):
                msl = slice(t * MT, (t + 1) * MT)
                for d in range(2):
                    av = [psB.tile([P, MT], FP, tag=f"av{i}", name=f"av{i}",
                                   bufs=2 - i) for i in range(2)]
                    racc = rsp.tile([P, MT], FP, tag="racc", name="racc")
                    racc_bf = rsp.tile([P, MT], BF, tag="raccbf",
                                       name="racc_bf")
                    stg_q = [None] * 8
                    for bb in range(9):
                        if bb < 8:
                            # scores + exp for burst bb
                            stg = stp.tile([P, 4 * MT], BF, tag="st",
                                           name="stg")
                            stg_q[bb] = stg
                            for half in range(2):
                                pt = psA.tile([P, 2, MT], FP, tag="pt",
                                              name="pt")
                                for rr in range(2):
                                    rg = 2 * half + rr
                                    kj = 4 * bb + rg
                                    ksl = slice(kj * P, (kj + 1) * P)
                                    nc.tensor.matmul(
                                        pt[:, rr, :], kt[d][:, ksl],
                                        qr[d][:, msl],
                                        start=True, stop=True)
                                nc.scalar.activation(
                                    stg[:, half * 1024:(half + 1) * 1024],
                                    pt[:, :, :], _ACT.Exp)
                        if bb == 0 and pend_tail is not None:
                            pend_tail()
                            pend_tail = None
                        if bb == 1 and pend_yc is not None:
                            pend_yc()
                            pend_yc = None
                        if bb == 2 and pend_stats is not None:
                            pend_stats()
                            pend_stats = None
                        if bb >= 1:
                            # AV + rowsum for burst bb-1
                            b = bb - 1
                            stg = stg_q[b]
                            for rg in range(4):
                                j = 4 * b + rg
                                ssl = slice(rg * MT, (rg + 1) * MT)
                                for cc in range(2):
                                    nc.tensor.matmul(
                                        av[cc],
                                        vt[d][j][:, cc * P:(cc + 1) * P],
                                        stg[:, ssl],
                                        start=(b == 0 and rg == 0),
                                        stop=(b == 7 and rg == 3))
                            t1 = rsp.tile([P, 2 * MT], BF, tag="t1",
                                          name="t1")
                            nc.vector.tensor_add(t1[:], stg[:, 0:1024],
                                                 stg[:, 1024:2048])
                            t2 = rsp.tile([P, MT], BF, tag="t2", name="t2")
                            nc.vector.tensor_add(t2[:], t1[:, 0:MT],
                                                 t1[:, MT:2 * MT])
                            if b == 0:
                                nc.gpsimd.tensor_copy(racc[:], t2[:])
                            else:
                                nc.gpsimd.tensor_add(
                                    racc_bf[:] if b == 7 else racc[:],
                                    racc[:], t2[:])
                    pend_tail = make_tail(t, d, av, racc_bf)
                pend_yc = make_yc(t)
                pend_stats = make_stats(t)

            pend_tail()
            pend_yc()
            pend_stats()

            # ---- BN: collapse partials, AllReduce, normalize ----
            stats = p_small.tile([P, 4], FP, tag="stats", name="stats")
            nc.vector.reduce_sum(stats[:, 0:2], ssum[:],
                                 axis=mybir.AxisListType.X)
            nc.vector.reduce_sum(stats[:, 2:4], ssq[:],
                                 axis=mybir.AxisListType.X)
            cc_in = dram.tile([P, 4], FP, name="cc_in")
            cc_out = dram.tile([P, 4], FP, name="cc_out")
            nc.sync.dma_start(cc_in[:], stats[:])
            nc.gpsimd.collective_compute(
                "AllReduce", _ALU.add,
                replica_groups=[list(range(NCORES))],
                ins=[cc_in.opt()], outs=[cc_out.opt()])
            ar = p_small.tile([P, 4], FP, tag="ar", name="ar")
            nc.sync.dma_start(ar[:], cc_out[:])

            inv_n = 1.0 / BN_COUNT
            yo = yout.rearrange("(o p) m -> p o m", p=P)
            scales, shifts = [], []
            for cc in range(2):
                mean = p_small.tile([P, 1], FP, tag="bn", name="mean")
                ex2 = p_small.tile([P, 1], FP, tag="bn", name="ex2")
                var = p_small.tile([P, 1], FP, tag="bn", name="var")
                nc.vector.tensor_scalar_mul(mean[:], ar[:, cc:cc + 1], inv_n)
                nc.vector.tensor_scalar_mul(ex2[:], ar[:, 2 + cc:3 + cc],
                                            inv_n)
                nc.vector.tensor_tensor(var[:], mean[:], mean[:], _ALU.mult)
                nc.vector.tensor_sub(var[:], ex2[:], var[:])
                sd = p_small.tile([P, 1], FP, tag="bn", name="sd")
                nc.vector.tensor_scalar_add(var[:], var[:], BN_EPS)
                nc.scalar.activation(sd[:], var[:], _ACT.Sqrt)
                rstd = p_small.tile([P, 1], FP, tag="bn", name="rstd")
                nc.vector.reciprocal(rstd[:], sd[:])
                scale = p_small.tile([P, 1], FP, tag="bnp", name="scale")
                nc.vector.tensor_tensor(scale[:], gb_sb[:, cc:cc + 1],
                                        rstd[:], _ALU.mult)
                shift = p_small.tile([P, 1], FP, tag="bnp", name="shift")
                nc.vector.tensor_tensor(shift[:], mean[:], scale[:],
                                        _ALU.mult)
                nc.vector.tensor_sub(shift[:], gb_sb[:, 2 + cc:3 + cc],
                                     shift[:])
                scales.append(scale)
                shifts.append(shift)
            for q in range(2):
                qsl = slice(q * 1024, (q + 1) * 1024)
                for cc in range(2):
                    nc.vector.tensor_scalar(
                        out=y_acc[cc][:, qsl], in0=y_acc[cc][:, qsl],
                        scalar1=scales[cc][:], scalar2=shifts[cc][:],
                        op0=_ALU.mult, op1=_ALU.add)
                    eng = nc.sync if cc == 0 else nc.scalar
                    eng.dma_start(yo[:, cc, qsl], y_acc[cc][:, qsl])

    nc.compile()
    return nc


def _get_program():
    global _PROGRAM
    if _PROGRAM is None:
        _PROGRAM = _build_program()
    return _PROGRAM


def _make_in_maps(inputs):
    BF_NP = mybir.dt.np(mybir.dt.bfloat16)
    f_p = np.ascontiguousarray(
        np.asarray(inputs["f_p"], np.float32).reshape(4, C, N))
    f_v = np.ascontiguousarray(
        np.asarray(inputs["f_v"], np.float32).reshape(4, C, N))

    def T(x):
        return np.ascontiguousarray(np.asarray(x, np.float32).T)

    w_out = np.asarray(inputs["w_out"], np.float32)
    bv_v = np.asarray(inputs["bv_v"], np.float32)
    bv_p = np.asarray(inputs["bv_p"], np.float32)
    # wv-bias terms of the cross contributions, folded into one vector.
    yb = w_out[:, 2 * C:3 * C] @ bv_v + w_out[:, 3 * C:] @ bv_p
    def pack_w(x, o):
        # [C_in, m] transposed weight -> per-partition [P, o*m] block
        t = T(x).astype(np.float32).reshape(o, P, -1).transpose(1, 0, 2)
        return t.reshape(P, -1)

    wcols = [
        pack_w(inputs["wq_p"], 2), pack_w(inputs["wq_v"], 2),
        pack_w(inputs["wk_v"], 2), pack_w(inputs["wk_p"], 2),
        pack_w(inputs["wv_v"], 2), pack_w(inputs["wv_p"], 2),
        pack_w(w_out[:, :2 * C], 4), pack_w(w_out[:, 2 * C:], 4),
    ]
    wpack = np.concatenate(wcols, axis=1).astype(BF_NP)
    biasq = np.stack(
        [np.tile(np.asarray(inputs[k], np.float32), 4)
         for k in ("bq_p", "bk_v", "bq_v", "bk_p")], axis=1)
    gamma = np.asarray(inputs["gamma"], np.float32)
    beta = np.asarray(inputs["beta"], np.float32)
    bpack = np.concatenate(
        [biasq, np.stack([yb[:P], yb[P:]], axis=1),
         np.stack([gamma[:P], gamma[P:], beta[:P], beta[P:]], axis=1)],
        axis=1).astype(np.float32)
    shared = {
        "wpack": np.ascontiguousarray(wpack),
        "bpack": np.ascontiguousarray(bpack),
    }
    in_maps = []
    for core in range(NCORES):
        b, h = divmod(core, 2)
        # roll so this core's query half sits at columns [0, 2048); K/V use
        # the full (permuted) range -- softmax/AV are key-order-invariant.
        kv1 = np.ascontiguousarray(
            np.roll(f_p[b], -h * M, axis=1).astype(BF_NP))
        kv0 = np.ascontiguousarray(
            np.roll(f_v[b], -h * M, axis=1).astype(BF_NP))
        in_maps.append({"kv0": kv0, "kv1": kv1, **shared})
    return in_maps


def _assemble(results):
    out = np.empty((4, C, N), np.float32)
    for core in range(NCORES):
        b, h = divmod(core, 2)
        out[b][:, h * M:(h + 1) * M] = results[core]["y"]
    return out.reshape(4, C, 64, 64)


def _run(inputs, **kwargs):
    nc = _get_program()
    in_maps = _make_in_maps(inputs)
    res = bass_utils.run_bass_kernel_spmd(
        nc, in_maps, core_ids=list(range(NCORES)), **kwargs)
    return _assemble(res.results), res


def kernel(**inputs):
    out, _ = _run(inputs)
    return out


# revision 21
# speedup vs baseline: 1.0972x; 1.0014x over previous
"""Trainium2 Bass kernel for nn_CrossAttentionExpert.

Problem (hardcoded shapes): B=4, C=256, H=W=64 (N=4096), C8=32.
  cross_p2v = attn(q=wq_p@f_p, k=wk_v@f_v, v=wv_v@f_v)
  cross_v2p = attn(q=wq_v@f_v, k=wk_p@f_p, v=wv_p@f_p)
  out = BN(w_out @ concat([f_p, f_v, cross_p2v, cross_v2p]))  (training BN)

Sharding: 8 cores = (batch b, spatial half h).  Each core computes both
attention directions for its 2048 query positions (keys/values span all
4096 positions of its batch), the fused 1x1 output conv, and BN with a
[128,4] fp32 AllReduce of per-channel sum/sumsq across all 8 cores.

Layout: scores are computed transposed, S^T[n,m] (n=key on partitions,
m=query on free axis) so the exp'd probabilities feed the V^T matmul
moving operand directly -- no on-chip transposes.  Softmax skips the
max-subtraction (logits are O(25); exp fits fp32/bf16 range with huge
margin for this problem's 0.05-scaled weights); 1/rowsum is applied
after the V-matmul via a PE outer-product broadcast.

Perf structure (v7; 682us baseline -> ~370-390us measured, the spread
is cross-core throttle/collective jitter -- the compute phase is
~315us):
 - Everything on the PE is bf16 (inputs, weights, qr/kt/stg/vt/cross):
   1 cycle/row, 2x faster LDWEIGHTS, half the SBUF and half the input
   DMA.  End-to-end rel err ~8.9e-3 vs the 2e-2 gate (validated in
   numpy and on HW).
 - Software pipeline: AV matmuls of burst b-1 are emitted after the
   score matmuls of burst b, so the in-order PE queue never blocks on
   the ACT exp of the current burst.  Tail work (rowsum collapse,
   1/rowsum, cross muls), the fused output conv, and BN stat partials
   of tile t are emitted inside tile t+1's burst stream for the same
   reason.  av0 is double-buffered so the next (t,d)'s accumulation
   starts before the previous tail drains.
 - Rowsum: bf16 pairwise adds on DVE (t1/t2), fp32 burst accumulation
   on the Pool engine (NOT gpsimd tensor_scalar -- that lowers to DSP
   software and takes ~170us), final value rounded to bf16 so the
   collapse/broadcast matmuls run 1-cycle/row (f32r can't: codegen
   rejects degenerate-stationary f32r matmuls).  1/x via the custom-DVE
   reciprocal_approx_fast (5x faster than InstReciprocal).
 - PSUM budget (8 banks): pt 2x2 banks, av0 2, av1 1, misc 1.
 - All conv weights ship as ONE packed [128,3328] bf16 DMA on the
   scalar queue (kv1 follows it; kv0 on the sync queue), and convs are
   emitted in data-arrival order, so the PE starts ~0 and the whole
   6MB input load hides under the projection matmuls.  Key chunk j of
   burst b is 4*b+rg (burst <-> K-conv sub b, V-conv j 4b..4b+3), so
   the first attention bursts only need the first key quarter and can
   start while later quarters still stream in.
 - BN channel-sums ride the output-conv PSUM->SBUF copy via ACT
   accum_out; sum-of-squares is a DVE scalar_tensor_tensor per m-tile.
   A dummy warmup AllReduce absorbs the collective's first-use cost;
   the real [128,4] AllReduce (~25us, latency-bound) is the only
   exposed tail besides normalize+writeback.
 - Known limits: the PE duty-cycle throttles to ~1.2GHz under
   sustained load (util_limit 50%), which caps the AV+scores stream at
   ~290us busy; fp8 DoubleRow would halve AV but needs a per-query max
   subtraction whose cheap bounds are too loose (validated: NaN/6%+
   error in numpy), so it is not used.
"""

import numpy as np

import concourse.bass as bass
import concourse.mybir as mybir
import concourse.tile as tile
from concourse import bacc, bass_utils

FP = mybir.dt.float32
FR = mybir.dt.float32r
BF = mybir.dt.bfloat16
P = 128
C = 256
C8 = 32
N = 4096          # full spatial positions per batch
M = 2048          # local query positions per core
NMT = 4           # m-tiles of 512
MT = 512
NCORES = 8
BN_EPS = 1e-5
BN_COUNT = 4 * 4096  # B * H * W

_ALU = mybir.AluOpType
_ACT = mybir.ActivationFunctionType

_PROGRAM = None


def _build_program():
    nc = bacc.Bacc("TRN2", target_bir_lowering=False, debug=False,
                   num_devices=NCORES)

    # ---- DRAM I/O ----
    kv = [nc.dram_tensor(f"kv{d}", [C, N], BF, kind="ExternalInput").ap()
          for d in range(2)]
    # all conv weights packed into one [P, 3328] bf16 tensor (one DMA):
    # cols: wq0(64) wq1(64) wk0(64) wk1(64) wv0(512) wv1(512)
    #       wout(1024) woutc(1024)
    wpack = nc.dram_tensor("wpack", [P, 3328], BF,
                           kind="ExternalInput").ap()
    bpack = nc.dram_tensor("bpack", [P, 10], FP, kind="ExternalInput").ap()
    yout = nc.dram_tensor("y", [C, M], FP, kind="ExternalOutput").ap()

    with tile.TileContext(nc) as tc:
        with (
            nc.allow_low_precision(
                reason="bf16 attention intermediates; "
                       "end-to-end rel err ~4.5e-3 vs 2e-2 gate"),
            tc.tile_pool(name="consts", bufs=1) as consts,
            tc.tile_pool(name="big", bufs=1) as big,
            tc.tile_pool(name="vt", bufs=64) as vtp,
            tc.tile_pool(name="st", bufs=3) as stp,
            tc.tile_pool(name="rs", bufs=3) as rsp,
            tc.tile_pool(name="cross", bufs=4) as p_cross,
            tc.tile_pool(name="small", bufs=4) as p_small,
            tc.tile_pool(name="psA", bufs=2, space="PSUM") as psA,
            tc.tile_pool(name="psB", bufs=1, space="PSUM") as psB,
            tc.tile_pool(name="psC", bufs=1, space="PSUM") as psC,
            tc.tile_pool(name="dram", bufs=1, space="DRAM") as dram,
        ):
            # ---- all weights in one DMA, biases in another ----
            wpack_sb = consts.tile([P, 3328], BF, name="wpacksb")
            nc.scalar.dma_start(wpack_sb[:], wpack[:])
            bpack_sb = consts.tile([P, 10], FP, name="bpacksb")
            nc.scalar.dma_start(bpack_sb[:], bpack[:])

            def wview(off, o, m):
                return wpack_sb[:, off:off + o * m].rearrange(
                    "p (o m) -> p o m", o=o)

            wq_sb = [wview(64 * d, 2, C8) for d in range(2)]
            wk_sb = [wview(128 + 64 * d, 2, C8) for d in range(2)]
            wv_sb = [wview(256 + 512 * d, 2, C) for d in range(2)]
            wout_sb = wview(1280, 4, C)
            woutc_sb = wview(2304, 4, C)
            biasq_sb = bpack_sb[:, 0:4]
            ybias_sb = bpack_sb[:, 4:6]
            gb_sb = bpack_sb[:, 6:10]

            ones_col = consts.tile([P, 1], BF, name="ones_col")
            nc.vector.memset(ones_col[:], 1.0)
            ones_row = consts.tile([1, P], BF, name="ones_row")
            nc.vector.memset(ones_row[:], 1.0)

            # ---- kv loads, position-major so convs can start early ----
            kv_sb = [big.tile([P, 2, N], BF, name=f"kvsb{d}")
                     for d in range(2)]
            for d in range(2):
                src = kv[d].rearrange("(o p) n -> p o n", p=P)
                eng = nc.sync if d == 0 else nc.scalar
                for h in range(2):
                    sl = slice(h * M, (h + 1) * M)
                    for o in range(2):
                        eng.dma_start(kv_sb[d][:, o, sl], src[:, o, sl])

            # warm up the collective path while convs run; result unused.
            warm_in = dram.tile([P, 4], FP, name="warm_in")
            warm_out = dram.tile([P, 4], FP, name="warm_out")
            nc.gpsimd.collective_compute(
                "AllReduce", _ALU.add,
                replica_groups=[list(range(NCORES))],
                ins=[warm_in.opt()], outs=[warm_out.opt()])

            # ---- persistent activations ----
            qr = [big.tile([32, M], BF, name=f"qr{d}") for d in range(2)]
            kt = [big.tile([32, N], BF, name=f"kt{d}") for d in range(2)]
            y_acc = [big.tile([P, M], FP, name=f"yacc{cc}") for cc in range(2)]
            vt = [[], []]

            # ---- projections, in DMA-arrival order ----
            # dir0: q from kv1 (f_p), k/v from kv0 (f_v); dir1 swapped.
            def k_conv(d, subs):
                kkv = kv_sb[d]
                for sub in subs:
                    nsl = slice(sub * MT, (sub + 1) * MT)
                    ps = psA.tile([32, MT], FP, tag="pt", name="kps")
                    for kc in range(2):
                        nc.tensor.matmul(
                            ps, wk_sb[d][:, kc, :], kkv[:, kc, nsl],
                            start=(kc == 0), stop=(kc == 1))
                    nc.scalar.activation(
                        kt[d][:, nsl], ps, _ACT.Identity,
                        bias=biasq_sb[0:32, 2 * d + 1:2 * d + 2])

            def v_conv(d, js):
                kkv = kv_sb[d]
                for j in js:
                    ps = psA.tile([P, C], FP, tag="pt", name="vps")
                    for kc in range(2):
                        nc.tensor.matmul(
                            ps, kkv[:, kc, j * P:(j + 1) * P],
                            wv_sb[d][:, kc, :],
                            start=(kc == 0), stop=(kc == 1))
                    v = vtp.tile([P, C], BF, tag="vt", name="vtt")
                    nc.vector.tensor_copy(v[:], ps)
                    vt[d].append(v)

            def q_conv(d):
                qkv = kv_sb[1 - d]
                for t in range(NMT):
                    msl = slice(t * MT, (t + 1) * MT)
                    ps = psA.tile([32, MT], FP, tag="pt", name="qps")
                    for kc in range(2):
                        nc.tensor.matmul(
                            ps, wq_sb[d][:, kc, :], qkv[:, kc, msl],
                            start=(kc == 0), stop=(kc == 1))
                    nc.scalar.activation(qr[d][:, msl], ps, _ACT.Identity,
                                         bias=biasq_sb[0:32, 2 * d:2 * d + 1])

            k_conv(0, [0, 1]); v_conv(0, range(0, 8))
            k_conv(0, [2, 3]); v_conv(0, range(8, 16))
            q_conv(1)          # reads kv0 first half
            q_conv(0)          # reads kv1 first half
            k_conv(0, [4, 5]); v_conv(0, range(16, 24))
            k_conv(0, [6, 7]); v_conv(0, range(24, 32))
            for h in range(2):
                k_conv(1, range(4 * h, 4 * h + 4))
                v_conv(1, range(16 * h, 16 * h + 16))

            # ---- BN stat partials, accumulated per m-tile ----
            ssum = p_small.tile([P, 2, NMT], FP, tag="ssum", name="ssum")
            ssq = p_small.tile([P, 2, NMT], FP, tag="ssq", name="ssq")

            # ---- attention + fused output conv ----
            # Tail/yc/stats of tile t are emitted inside tile t+1's burst
            # stream so the in-order PE queue never blocks on them.
            crs_all = {}

            def make_tail(t, d, av, racc_bf, last=False):
                pool = psA if last else psC
                tag = "pt" if last else "misc"

                def emit():
                    rsum_ps = pool.tile([1, MT], FP, tag=tag, name="rsum")
                    nc.tensor.matmul(rsum_ps, ones_col[:], racc_bf[:],
                                     start=True, stop=True)
                    rs_bf = p_small.tile([1, MT], BF, tag="rsbf",
                                         name="rsbf")
                    nc.vector.tensor_copy(rs_bf[:], rsum_ps)
                    rbc_ps = pool.tile([P, MT], FP, tag=tag, name="rbc")
                    nc.tensor.matmul(rbc_ps, ones_row[:], rs_bf[:],
                                     start=True, stop=True)
                    rbc = p_cross.tile([P, MT], FP, tag="rbc", name="rbc_sb")
                    nc.vector.reciprocal_approx_fast(out=rbc[:], in_=rbc_ps)
                    for cc in range(2):
                        cross = p_cross.tile([P, MT], BF, tag="cross",
                                             name="cross")
                        nc.vector.tensor_mul(cross[:], av[cc], rbc[:])
                        crs_all[(t, d, cc)] = cross
                return emit

            def make_yc(t):
                msl = slice(t * MT, (t + 1) * MT)

                def emit():
                    for oc in range(2):
                        ocs = slice(oc * P, (oc + 1) * P)
                        yc = psC.tile([P, MT], FP, tag="misc", name="yc")
                        nc.tensor.matmul(yc, wout_sb[:, 0, ocs],
                                         kv_sb[1][:, 0, msl],
                                         start=True, stop=False)
                        nc.tensor.matmul(yc, wout_sb[:, 1, ocs],
                                         kv_sb[1][:, 1, msl],
                                         start=False, stop=False)
                        nc.tensor.matmul(yc, wout_sb[:, 2, ocs],
                                         kv_sb[0][:, 0, msl],
                                         start=False, stop=False)
                        nc.tensor.matmul(yc, wout_sb[:, 3, ocs],
                                         kv_sb[0][:, 1, msl],
                                         start=False, stop=False)
                        for d in range(2):
                            for cc in range(2):
                                nc.tensor.matmul(
                                    yc, woutc_sb[:, 2 * d + cc, ocs],
                                    crs_all[(t, d, cc)][:],
                                    start=False,
                                    stop=(d == 1 and cc == 1))
                        nc.scalar.activation(y_acc[oc][:, msl], yc,
                                             _ACT.Identity,
                                             bias=ybias_sb[:, oc:oc + 1],
                                             accum_out=ssum[:, oc, t:t + 1])
                        sq = p_small.tile([P, MT], BF, tag="sq", name="sq",
                                          bufs=2)
                        nc.vector.scalar_tensor_tensor(
                            out=sq[:], in0=y_acc[oc][:, msl], scalar=1.0,
                            in1=y_acc[oc][:, msl],
                            op0=_ALU.mult, op1=_ALU.mult,
                            accum_out=ssq[:, oc, t:t + 1])
                return emit

            def make_stats(t):
                def emit():
                    pass
                return emit

            pend_tail = pend_yc = pend_stats = None
            for t in range(NMT):
                msl = slice(t * MT, (t + 1) * MT)
                for d in range(2):
                    av = [psB.tile([P, MT], FP, tag=f"av{i}", name=f"av{i}",
                                   bufs=2 - i) for i in range(2)]
                    racc = rsp.tile([P, MT], FP, tag="racc", name="racc")
                    racc_bf = rsp.tile([P, MT], BF, tag="raccbf",
                                       name="racc_bf")
                    stg_q = [None] * 8
                    for bb in range(9):
                        if bb < 8:
                            # scores + exp for burst bb
                            stg = stp.tile([P, 4 * MT], BF, tag="st",
                                           name="stg")
                            stg_q[bb] = stg
                            for half in range(2):
                                pt = psA.tile([P, 2, MT], FP, tag="pt",
                                              name="pt")
                                for rr in range(2):
                                    rg = 2 * half + rr
                                    kj = 4 * bb + rg
                                    ksl = slice(kj * P, (kj + 1) * P)
                                    nc.tensor.matmul(
                                        pt[:, rr, :], kt[d][:, ksl],
                                        qr[d][:, msl],
                                        start=True, stop=True)
                                nc.scalar.activation(
                                    stg[:, half * 1024:(half + 1) * 1024],
                                    pt[:, :, :], _ACT.Exp)
                        if bb == 0 and pend_tail is not None:
                            pend_tail()
                            pend_tail = None
                        if bb == 1 and pend_yc is not None:
                            pend_yc()
                            pend_yc = None
                        if bb == 2 and pend_stats is not None:
                            pend_stats()
                            pend_stats = None
                        if bb >= 1:
                            # AV + rowsum for burst bb-1
                            b = bb - 1
                            stg = stg_q[b]
                            for rg in range(4):
                                j = 4 * b + rg
                                ssl = slice(rg * MT, (rg + 1) * MT)
                                for cc in range(2):
                                    nc.tensor.matmul(
                                        av[cc],
                                        vt[d][j][:, cc * P:(cc + 1) * P],
                                        stg[:, ssl],
                                        start=(b == 0 and rg == 0),
                                        stop=(b == 7 and rg == 3))
                            t1 = rsp.tile([P, 2 * MT], BF, tag="t1",
                                          name="t1")
                            nc.vector.tensor_add(t1[:], stg[:, 0:1024],
                                                 stg[:, 1024:2048])
                            t2 = rsp.tile([P, MT], BF, tag="t2", name="t2")
                            nc.vector.tensor_add(t2[:], t1[:, 0:MT],
                                                 t1[:, MT:2 * MT])
                            if b == 0:
                                nc.gpsimd.tensor_copy(racc[:], t2[:])
                            else:
                                nc.gpsimd.tensor_add(
                                    racc_bf[:] if b == 7 else racc[:],
                                    racc[:], t2[:])
                    pend_tail = make_tail(t, d, av, racc_bf,
                                          last=(t == NMT - 1 and d == 1))
                pend_yc = make_yc(t)
                pend_stats = make_stats(t)

            pend_tail()
            pend_yc()
            pend_stats()

            # ---- BN: collapse partials, AllReduce, normalize ----
            stats = p_small.tile([P, 4], FP, tag="stats", name="stats")
            nc.vector.reduce_sum(stats[:, 0:2], ssum[:],
                                 axis=mybir.AxisListType.X)
            nc.vector.reduce_sum(stats[:, 2:4], ssq[:],
                                 axis=mybir.AxisListType.X)
            cc_in = dram.tile([P, 4], FP, name="cc_in")
            cc_out = dram.tile([P, 4], FP, name="cc_out")
            nc.sync.dma_start(cc_in[:], stats[:])
            nc.gpsimd.collective_compute(
                "AllReduce", _ALU.add,
                replica_groups=[list(range(NCORES))],
                ins=[cc_in.opt()], outs=[cc_out.opt()])
            ar = p_small.tile([P, 4], FP, tag="ar", name="ar")
            nc.sync.dma_start(ar[:], cc_out[:])

            inv_n = 1.0 / BN_COUNT
            yo = yout.rearrange("(o p) m -> p o m", p=P)
            scales, shifts = [], []
            for cc in range(2):
                mean = p_small.tile([P, 1], FP, tag="bn", name="mean")
                ex2 = p_small.tile([P, 1], FP, tag="bn", name="ex2")
                var = p_small.tile([P, 1], FP, tag="bn", name="var")
                nc.vector.tensor_scalar_mul(mean[:], ar[:, cc:cc + 1], inv_n)
                nc.vector.tensor_scalar_mul(ex2[:], ar[:, 2 + cc:3 + cc],
                                            inv_n)
                nc.vector.tensor_tensor(var[:], mean[:], mean[:], _ALU.mult)
                nc.vector.tensor_sub(var[:], ex2[:], var[:])
                sd = p_small.tile([P, 1], FP, tag="bn", name="sd")
                nc.vector.tensor_scalar_add(var[:], var[:], BN_EPS)
                nc.scalar.activation(sd[:], var[:], _ACT.Sqrt)
                rstd = p_small.tile([P, 1], FP, tag="bn", name="rstd")
                nc.vector.reciprocal(rstd[:], sd[:])
                scale = p_small.tile([P, 1], FP, tag="bnp", name="scale")
                nc.vector.tensor_tensor(scale[:], gb_sb[:, cc:cc + 1],
                                        rstd[:], _ALU.mult)
                shift = p_small.tile([P, 1], FP, tag="bnp", name="shift")
                nc.vector.tensor_tensor(shift[:], mean[:], scale[:],
                                        _ALU.mult)
                nc.vector.tensor_sub(shift[:], gb_sb[:, 2 + cc:3 + cc],
                                     shift[:])
                scales.append(scale)
                shifts.append(shift)
            for q in range(2):
                qsl = slice(q * 1024, (q + 1) * 1024)
                for cc in range(2):
                    nc.vector.tensor_scalar(
                        out=y_acc[cc][:, qsl], in0=y_acc[cc][:, qsl],
                        scalar1=scales[cc][:], scalar2=shifts[cc][:],
                        op0=_ALU.mult, op1=_ALU.add)
                    eng = nc.sync if cc == 0 else nc.scalar
                    eng.dma_start(yo[:, cc, qsl], y_acc[cc][:, qsl])

    nc.compile()
    return nc


def _get_program():
    global _PROGRAM
    if _PROGRAM is None:
        _PROGRAM = _build_program()
    return _PROGRAM


def _make_in_maps(inputs):
    BF_NP = mybir.dt.np(mybir.dt.bfloat16)
    f_p = np.ascontiguousarray(
        np.asarray(inputs["f_p"], np.float32).reshape(4, C, N))
    f_v = np.ascontiguousarray(
        np.asarray(inputs["f_v"], np.float32).reshape(4, C, N))

    def T(x):
        return np.ascontiguousarray(np.asarray(x, np.float32).T)

    w_out = np.asarray(inputs["w_out"], np.float32)
    bv_v = np.asarray(inputs["bv_v"], np.float32)
    bv_p = np.asarray(inputs["bv_p"], np.float32)
    # wv-bias terms of the cross contributions, folded into one vector.
    yb = w_out[:, 2 * C:3 * C] @ bv_v + w_out[:, 3 * C:] @ bv_p
    def pack_w(x, o):
        # [C_in, m] transposed weight -> per-partition [P, o*m] block
        t = T(x).astype(np.float32).reshape(o, P, -1).transpose(1, 0, 2)
        return t.reshape(P, -1)

    wcols = [
        pack_w(inputs["wq_p"], 2), pack_w(inputs["wq_v"], 2),
        pack_w(inputs["wk_v"], 2), pack_w(inputs["wk_p"], 2),
        pack_w(inputs["wv_v"], 2), pack_w(inputs["wv_p"], 2),
        pack_w(w_out[:, :2 * C], 4), pack_w(w_out[:, 2 * C:], 4),
    ]
    wpack = np.concatenate(wcols, axis=1).astype(BF_NP)
    biasq = np.stack(
        [np.tile(np.asarray(inputs[k], np.float32), 4)
         for k in ("bq_p", "bk_v", "bq_v", "bk_p")], axis=1)
    gamma = np.asarray(inputs["gamma"], np.float32)
    beta = np.asarray(inputs["beta"], np.float32)
    bpack = np.concatenate(
        [biasq, np.stack([yb[:P], yb[P:]], axis=1),
         np.stack([gamma[:P], gamma[P:], beta[:P], beta[P:]], axis=1)],
        axis=1).astype(np.float32)
    shared = {
        "wpack": np.ascontiguousarray(wpack),
        "bpack": np.ascontiguousarray(bpack),
    }
    in_maps = []
    for core in range(NCORES):
        b, h = divmod(core, 2)
        # roll so this core's query half sits at columns [0, 2048); K/V use
        # the full (permuted) range -- softmax/AV are key-order-invariant.
        kv1 = np.ascontiguousarray(
            np.roll(f_p[b], -h * M, axis=1).astype(BF_NP))
        kv0 = np.ascontiguousarray(
            np.roll(f_v[b], -h * M, axis=1).astype(BF_NP))
        in_maps.append({"kv0": kv0, "kv1": kv1, **shared})
    return in_maps


def _assemble(results):
    out = np.empty((4, C, N), np.float32)
    for core in range(NCORES):
        b, h = divmod(core, 2)
        out[b][:, h * M:(h + 1) * M] = results[core]["y"]
    return out.reshape(4, C, 64, 64)


def _run(inputs, **kwargs):
    nc = _get_program()
    in_maps = _make_in_maps(inputs)
    res = bass_utils.run_bass_kernel_spmd(
        nc, in_maps, core_ids=list(range(NCORES)), **kwargs)
    return _assemble(res.results), res


def kernel(**inputs):
    out, _ = _run(inputs)
    return out
